# revision 10
# baseline (speedup 1.0000x reference)
"""Trainium2 Bass kernel for nn_CausalRecurrenceLayer.

Sharding: 8 cores = 4 batches x 2 sequence-halves. Device layout is
channel-major [c, t] for the conv/gate matmuls and the hardware scan
(tensor_tensor_scan); the output projection is emitted as [t, j] so it DMAs
directly into the [B, L, d] output.

Self-contained: hardcodes shapes B=4, L=4096, d=1024.
"""
import sys

sys.path.insert(0, "/opt/trn_rl_repo")

import numpy as np
import ml_dtypes

import concourse.bass as bass  # noqa: F401  (bass.ts used via slices)
import concourse.tile as tile
from concourse import bacc, mybir
from concourse import bass_utils

F32 = mybir.dt.float32
F32R = mybir.dt.float32r
BF16 = mybir.dt.bfloat16
AF = mybir.ActivationFunctionType
OP = mybir.AluOpType

B, L, D = 4, 4096, 1024
TH = L // 2      # per-core sequence extent
TT = 512         # time tile
NT = TH // TT    # 4 time tiles per core
P = 128
CB = D // P      # 8 channel blocks
EPS = 1e-6

_compiled = {}


def _build(trace=False):
    nc = bacc.Bacc("TRN2", target_bir_lowering=False, debug=False, num_devices=8)

    # ---- DRAM I/O ----
    x_d = nc.dram_tensor("x_sh", [D, TH + 3], F32, kind="ExternalInput").ap()
    wr_d = nc.dram_tensor("wrT", [D, D], BF16, kind="ExternalInput").ap()
    wi_d = nc.dram_tensor("wiT", [D, D], BF16, kind="ExternalInput").ap()
    wo_d = nc.dram_tensor("woT", [D, D], F32R, kind="ExternalInput").ap()
    # per-channel columns, laid out [128, CB]
    br_d = nc.dram_tensor("br_c", [P, CB], F32, kind="ExternalInput").ap()
    bi_d = nc.dram_tensor("bi_c", [P, CB], F32, kind="ExternalInput").ap()
    cb_d = nc.dram_tensor("cb_c", [P, CB], F32, kind="ExternalInput").ap()
    w0_d = nc.dram_tensor("w0_c", [P, CB], F32, kind="ExternalInput").ap()
    w1_d = nc.dram_tensor("w1_c", [P, CB], F32, kind="ExternalInput").ap()
    w2_d = nc.dram_tensor("w2_c", [P, CB], F32, kind="ExternalInput").ap()
    w3_d = nc.dram_tensor("w3_c", [P, CB], F32, kind="ExternalInput").ap()
    c1_d = nc.dram_tensor("c1_c", [P, CB], F32, kind="ExternalInput").ap()   # 8*ln(sigmoid(log_a))
    c2_d = nc.dram_tensor("c2_c", [P, CB], F32, kind="ExternalInput").ap()   # 2*c1
    tm_d = nc.dram_tensor("tmask", [P, 1], F32, kind="ExternalInput").ap()   # 1.0 iff second half
    y_d = nc.dram_tensor("y", [TH, D], F32, kind="ExternalOutput").ap()

    with tile.TileContext(nc) as tc:
        with (
            tc.tile_pool(name="wpool", bufs=1) as wpool,
            tc.tile_pool(name="sbuf", bufs=1) as sb,
            tc.tile_pool(name="store", bufs=1) as store,
            tc.tile_pool(name="psum", bufs=1, space="PSUM") as ps,
            tc.tile_pool(name="dram", bufs=1, space="DRAM") as dp,
        ):
            # ---- resident weights / constants ----
            wr_t, wi_t, wo_t = [], [], []
            for cb in range(CB):
                t = wpool.tile([P, D], BF16, tag=f"wr{cb}")
                nc.sync.dma_start(t[:], wr_d[cb * P:(cb + 1) * P, :])
                wr_t.append(t)
                t = wpool.tile([P, D], BF16, tag=f"wi{cb}")
                nc.sync.dma_start(t[:], wi_d[cb * P:(cb + 1) * P, :])
                wi_t.append(t)
                t = wpool.tile([P, D], F32R, tag=f"wo{cb}")
                nc.sync.dma_start(t[:], wo_d[cb * P:(cb + 1) * P, :])
                wo_t.append(t)
            br_t = wpool.tile([P, CB], F32, tag="br")
            nc.sync.dma_start(br_t[:], br_d)
            bi_t = wpool.tile([P, CB], F32, tag="bi")
            nc.sync.dma_start(bi_t[:], bi_d)
            cb_t = wpool.tile([P, CB], F32, tag="cbias")
            nc.sync.dma_start(cb_t[:], cb_d)
            wk_t = []
            for k, wd in enumerate((w0_d, w1_d, w2_d, w3_d)):
                t = wpool.tile([P, CB], F32, tag=f"wk{k}")
                nc.sync.dma_start(t[:], wd)
                wk_t.append(t)
            c1_t = wpool.tile([P, CB], F32, tag="c1")
            nc.sync.dma_start(c1_t[:], c1_d)
            c2_t = wpool.tile([P, CB], F32, tag="c2")
            nc.sync.dma_start(c2_t[:], c2_d)
            tm_t = wpool.tile([P, 1], F32, tag="tm")
            nc.sync.dma_start(tm_t[:], tm_d)
            eps_t = wpool.tile([P, 1], F32, tag="eps")
            nc.vector.memset(eps_t[:], EPS)

            # persistent per-half stores
            am1_t = [store.tile([P, TH], BF16, tag=f"am1_{cb}", name=f"am1_{cb}") for cb in range(CB)]
            hl_sb = store.tile([P, CB], F32, tag="hl")       # local h_last columns
            bb_spill = dp.tile([D, TH], F32, tag="bbsp")     # gated-input spill
            ag_in = dp.tile([1, D], F32, tag="ag_in")
            ag_out = dp.tile([2, D], F32, tag="ag_out")

            # =========== PHASE A: conv + gates + pass-1 scan ===========
            scan1_prev = [None] * CB
            for t0 in range(NT):
                # conv -> xc for all channel blocks of this t-tile
                xc_t = []
                xcb_t = []
                for cb in range(CB):
                    xt = sb.tile([P, TT + 3], F32, tag="xraw", bufs=2)
                    nc.sync.dma_start(xt[:], x_d[cb * P:(cb + 1) * P, t0 * TT:t0 * TT + TT + 3])
                    acc = sb.tile([P, TT], F32, tag="cacc", bufs=2)
                    # (x[t]*w3 + conv_b)
                    nc.vector.tensor_scalar(acc[:], xt[:, 3:3 + TT],
                                            wk_t[3][:, cb:cb + 1], cb_t[:, cb:cb + 1],
                                            OP.mult, OP.add)
                    acc2 = sb.tile([P, TT], F32, tag="cacc2", bufs=1)
                    nc.vector.scalar_tensor_tensor(acc2[:], xt[:, 2:2 + TT],
                                                   wk_t[2][:, cb:cb + 1], acc[:],
                                                   OP.mult, OP.add)
                    acc3 = sb.tile([P, TT], F32, tag="cacc3", bufs=1)
                    nc.vector.scalar_tensor_tensor(acc3[:], xt[:, 1:1 + TT],
                                                   wk_t[1][:, cb:cb + 1], acc2[:],
                                                   OP.mult, OP.add)
                    xc = sb.tile([P, TT], F32, tag="xc", bufs=9)
                    nc.vector.scalar_tensor_tensor(xc[:], xt[:, 0:TT],
                                                   wk_t[0][:, cb:cb + 1], acc3[:],
                                                   OP.mult, OP.add)
                    xcb = sb.tile([P, TT], BF16, tag="xcb", bufs=9)
                    nc.gpsimd.tensor_copy(xcb[:], xc[:])
                    xc_t.append(xc)
                    xcb_t.append(xcb)

                # gate matmuls + elementwise per output channel block
                for cb in range(CB):
                    r_ps = ps.tile([P, TT], F32, tag="r_ps", bufs=2)
                    i_ps = ps.tile([P, TT], F32, tag="i_ps", bufs=2)
                    for kb in range(CB):
                        nc.tensor.matmul(r_ps[:], wr_t[kb][:, cb * P:(cb + 1) * P],
                                         xcb_t[kb][:], start=(kb == 0), stop=(kb == CB - 1))
                    for kb in range(CB):
                        nc.tensor.matmul(i_ps[:], wi_t[kb][:, cb * P:(cb + 1) * P],
                                         xcb_t[kb][:], start=(kb == 0), stop=(kb == CB - 1))
                    sr = sb.tile([P, TT], F32, tag="sr", bufs=2)
                    nc.scalar.activation(sr[:], r_ps[:], AF.Sigmoid, bias=br_t[:, cb:cb + 1])
                    a_t = sb.tile([P, TT], F32, tag="a_t", bufs=2)
                    nc.scalar.activation(a_t[:], sr[:], AF.Exp, scale=c1_t[:, cb:cb + 1])
                    asq = sb.tile([P, TT], F32, tag="asq", bufs=1)
                    nc.scalar.activation(asq[:], sr[:], AF.Exp, scale=c2_t[:, cb:cb + 1])
                    # store a-1 in bf16 for pass 2
                    nc.vector.tensor_scalar_add(am1_t[cb][:, t0 * TT:(t0 + 1) * TT], a_t[:], -1.0)
                    scl = sb.tile([P, TT], F32, tag="scl", bufs=1)
                    nc.scalar.activation(scl[:], asq[:], AF.Sqrt, bias=1.0, scale=-1.0)
                    si = sb.tile([P, TT], F32, tag="si", bufs=2)
                    nc.scalar.activation(si[:], i_ps[:], AF.Sigmoid, bias=bi_t[:, cb:cb + 1])
                    b1 = sb.tile([P, TT], F32, tag="b1", bufs=1)
                    nc.vector.tensor_tensor(b1[:], si[:], scl[:], OP.mult)
                    bb = sb.tile([P, TT], F32, tag="bb", bufs=3)
                    nc.gpsimd.tensor_tensor(bb[:], b1[:], xc_t[cb][:], OP.mult)
                    nc.sync.dma_start(bb_spill[cb * P:(cb + 1) * P, t0 * TT:(t0 + 1) * TT], bb[:])
                    # pass-1 scan (local, zero initial), chained via carry cols
                    s1 = sb.tile([P, TT], F32, tag="s1", bufs=2)
                    init = 0.0 if t0 == 0 else scan1_prev[cb][:, 0:1]
                    nc.vector.tensor_tensor_scan(s1[:], a_t[:], bb[:], init, OP.mult, OP.add)
                    if t0 == NT - 1:
                        nc.vector.tensor_copy(hl_sb[:, cb:cb + 1], s1[:, TT - 1:TT])
                    else:
                        cy = sb.tile([P, 1], F32, tag=f"cy{cb}", name=f"cy{cb}", bufs=2)
                        nc.vector.tensor_copy(cy[:], s1[:, TT - 1:TT])
                        scan1_prev[cb] = cy

            # =========== collective: exchange local h_last ===========
            nc.sync.dma_start(ag_in[:].rearrange("one (cb p) -> p (one cb)", p=P), hl_sb[:])
            nc.gpsimd.collective_compute(
                "AllGather", OP.bypass,
                replica_groups=[[0, 1], [2, 3], [4, 5], [6, 7]],
                ins=[ag_in[:].opt()], outs=[ag_out[:].opt()],
            )
            g0 = store.tile([P, CB], F32, tag="g0")
            nc.sync.dma_start(g0[:], ag_out[0:1, :].rearrange("one (cb p) -> p (one cb)", p=P))
            init_c = store.tile([P, CB], F32, tag="init_c")
            nc.vector.tensor_scalar_mul(init_c[:], g0[:], tm_t[:, 0:1])

            # =========== PHASE B/D: true scan + out-proj + RMSNorm ===========
            h_prev = [None] * CB
            for t0 in range(NT):
                h_t = []
                for cb in range(CB):
                    bbr = sb.tile([P, TT], F32, tag="bbr", bufs=2)
                    nc.sync.dma_start(bbr[:], bb_spill[cb * P:(cb + 1) * P, t0 * TT:(t0 + 1) * TT])
                    ar = sb.tile([P, TT], F32, tag="ar", bufs=2)
                    nc.vector.tensor_scalar_add(ar[:], am1_t[cb][:, t0 * TT:(t0 + 1) * TT], 1.0)
                    h = sb.tile([P, TT], F32R, tag="h", bufs=9)
                    init = init_c[:, cb:cb + 1] if t0 == 0 else h_prev[cb][:, 0:1]
                    nc.vector.tensor_tensor_scan(h[:], ar[:], bbr[:], init, OP.mult, OP.add)
                    h_t.append(h)
                    if t0 != NT - 1:
                        hc = sb.tile([P, 1], F32, tag=f"hc{cb}", name=f"hc{cb}", bufs=2)
                        nc.vector.tensor_copy(hc[:], h[:, TT - 1:TT])
                        h_prev[cb] = hc
                # out projection per 128-row chunk, then RMS norm
                for ch in range(TT // P):
                    o_ps = ps.tile([P, D], F32, tag="o_ps", bufs=2)
                    for jh in range(2):
                        for kb in range(CB):
                            nc.tensor.matmul(
                                o_ps[:, jh * 512:(jh + 1) * 512],
                                h_t[kb][:, ch * P:(ch + 1) * P],
                                wo_t[kb][:, jh * 512:(jh + 1) * 512],
                                start=(kb == 0), stop=(kb == CB - 1))
                    sq0 = sb.tile([P, 512], F32, tag="sq0", bufs=1)
                    ss0 = sb.tile([P, 1], F32, tag="ss0", bufs=2)
                    nc.scalar.activation(sq0[:], o_ps[:, 0:512], AF.Square, accum_out=ss0[:])
                    sq1 = sb.tile([P, 512], F32, tag="sq1", bufs=1)
                    ss1 = sb.tile([P, 1], F32, tag="ss1", bufs=2)
                    nc.scalar.activation(sq1[:], o_ps[:, 512:1024], AF.Square, accum_out=ss1[:])
                    ssum = sb.tile([P, 1], F32, tag="ssum", bufs=2)
                    nc.vector.tensor_tensor(ssum[:], ss0[:], ss1[:], OP.add)
                    s = sb.tile([P, 1], F32, tag="s_rms", bufs=2)
                    nc.scalar.activation(s[:], ssum[:], AF.Sqrt, scale=1.0 / D, bias=eps_t[:, 0:1])
                    rinv = sb.tile([P, 1], F32, tag="rinv", bufs=2)
                    nc.vector.reciprocal(rinv[:], s[:])
                    y_sb = sb.tile([P, D], F32, tag="y_sb", bufs=2)
                    nc.vector.tensor_scalar_mul(y_sb[:, 0:512], o_ps[:, 0:512], rinv[:, 0:1])
                    nc.vector.tensor_scalar_mul(y_sb[:, 512:1024], o_ps[:, 512:1024], rinv[:, 0:1])
                    nc.sync.dma_start(y_d[t0 * TT + ch * P: t0 * TT + (ch + 1) * P, :], y_sb[:])

    nc.compile()
    return nc


def kernel(**inputs):
    x = np.asarray(inputs["x"], np.float32)
    conv_w = np.asarray(inputs["conv_w"], np.float32)
    conv_b = np.asarray(inputs["conv_b"], np.float32)
    W_r = np.asarray(inputs["W_r"], np.float32)
    b_r = np.asarray(inputs["b_r"], np.float32)
    W_i = np.asarray(inputs["W_i"], np.float32)
    b_i = np.asarray(inputs["b_i"], np.float32)
    log_a = np.asarray(inputs["log_a"], np.float32)
    W_out = np.asarray(inputs["W_out"], np.float32)
    gamma = np.asarray(inputs["gamma"], np.float32)
    assert x.shape == (B, L, D), x.shape

    if "nc" not in _compiled:
        _compiled["nc"] = _build()
    nc = _compiled["nc"]

    def col(v):  # [D] -> [128, CB] with column cb = v[cb*128:(cb+1)*128]
        return np.ascontiguousarray(v.reshape(CB, P).T).astype(np.float32)

    xT = np.ascontiguousarray(x.transpose(0, 2, 1))            # [B, D, L]
    wrT = np.ascontiguousarray(W_r.T).astype(ml_dtypes.bfloat16)
    wiT = np.ascontiguousarray(W_i.T).astype(ml_dtypes.bfloat16)
    woT = np.ascontiguousarray((W_out * gamma[:, None]).T).astype(np.float32)
    a_base = 1.0 / (1.0 + np.exp(-log_a.astype(np.float64)))
    c1 = (8.0 * np.log(a_base)).astype(np.float32)             # [D]
    common = {
        "wrT": wrT, "wiT": wiT, "woT": woT,
        "br_c": col(b_r), "bi_c": col(b_i), "cb_c": col(conv_b),
        "w0_c": col(conv_w[:, 0, 0]), "w1_c": col(conv_w[:, 0, 1]),
        "w2_c": col(conv_w[:, 0, 2]), "w3_c": col(conv_w[:, 0, 3]),
        "c1_c": col(c1), "c2_c": col(2.0 * c1),
    }
    in_maps = []
    for k in range(8):
        b, th = k // 2, k % 2
        xs = np.zeros((D, TH + 3), np.float32)
        lo = th * TH - 3
        if lo < 0:
            xs[:, 3:] = xT[b, :, 0:TH]
        else:
            xs[:] = xT[b, :, lo:lo + TH + 3]
        m = dict(common)
        m["x_sh"] = xs
        m["tmask"] = np.full((P, 1), float(th), np.float32)
        in_maps.append(m)

    import os
    trace = bool(int(os.environ.get("KERNEL_TRACE", "0")))
    kw = {}
    if trace:
        kw = dict(trace=True, trace_cores=list(range(8)))
    res = bass_utils.run_bass_kernel_spmd(nc, in_maps, core_ids=list(range(8)), **kw)
    _compiled["last_exec_time_ns"] = res.exec_time_ns

    out = np.empty((B, L, D), np.float32)
    for k in range(8):
        b, th = k // 2, k % 2
        out[b, th * TH:(th + 1) * TH, :] = res.results[k]["y"]
    return out
